# revision 2
# baseline (speedup 1.0000x reference)
"""IntersectionLoss Trainium2 kernel.

Math: loss_n = maskedmean_j relu(R + S*log(sum_i exp(-|t2_nj - t1_ni|^2/S) * m1_i + eps))
Key identity: |t2_j - t1_i|^2 = n2_j + n1_i - 2*t2_j.t1_i, so the inner sum is a
K=4 matmul G' = t2aug^T.T @ t1aug^T with augmented rows
  t1aug = [x, y, z, -n1_i/2 + (S/2)ln m1_i],  t2aug = [x, y, z, 1],
followed by ACT Exp with per-partition bias: exp((2/S)G' - n2_j/S) equals
exp(-d^2/S) * m1_i elementwise, and the L1-reduction rides on ACT's accum_out.
Each (128 j) x (2048 i) plane is one PE f32r stream + one ACT Exp stream
(flash-style: the pairwise matrix only ever exists in PSUM, 2 planes deep).

The tiny (N,2048,4) operand prep (augmented rows, transpose, bias) runs on host;
the 67M-element pairwise exp/matmul/reduce work runs on device. Sharding:
data-parallel over N=16 across 8 cores (2 batches per core). Final
log/relu/masked-mean over the (N,2048) accumulator runs on host in float64.
"""

import sys

sys.path.insert(0, "/opt/trn_rl_repo")

import numpy as np

import concourse.bass as bass
import concourse.tile as tile
from concourse import mybir
from concourse.bass_utils import run_bass_kernel_spmd

RADIUS = 1.0
SIGMA = 2.5
EPSILON = 1e-12

N, L1, L2 = 16, 2048, 2048
NCORES = 8
NB = N // NCORES  # batches per core
P = 128
A = L2 // P  # 16 j-tiles per batch
F32 = mybir.dt.float32
F32R = mybir.dt.float32r
AF = mybir.ActivationFunctionType

_CACHE = {}


def _build_program():
    nc = bass.Bass()
    # taug[b, k, s, i]: s=0 -> t1aug row k, s=1 -> t2aug row k (i in 0..2047)
    taug_d = nc.declare_dram_parameter("taug", (NB, 4, 2, L1), F32R, isOutput=False)
    bias_d = nc.declare_dram_parameter("bias", (P, NB * A), F32, isOutput=False)
    acc_d = nc.declare_dram_parameter("acc", (P, NB * A), F32, isOutput=True)

    with tile.TileContext(nc) as tc:
        with (
            tc.tile_pool(name="consts", bufs=1) as consts,
            tc.tile_pool(name="sb", bufs=2) as sb,
            tc.tile_pool(name="ps", bufs=2, space="PSUM") as ps,
        ):
            # one DMA for all matmul operands: tT[k, (b s i)]
            tT = consts.tile([4, NB * 2 * L1], F32R)
            nc.sync.dma_start(
                out=tT.rearrange("k (b s i) -> k b s i", b=NB, s=2),
                in_=taug_d.rearrange("b k s i -> k b s i"),
            )
            bias_all = consts.tile([P, NB * A], F32)
            nc.sync.dma_start(out=bias_all[:], in_=bias_d[:])
            # absorb the bias-DMA wait on ACT so the Exp instrs carry <=1 wait
            scratch = consts.tile([P, NB * A], F32)
            nc.scalar.copy(scratch[:], bias_all[:])

            acc_sb = sb.tile([P, NB * A], F32, tag="acc")
            for b in range(NB):
                for jt in range(A):
                    g = ps.tile([P, L1], F32, tag="ps")
                    lhsT = tT[:, (2 * b + 1) * L1 + jt * P : (2 * b + 1) * L1 + (jt + 1) * P]
                    for it in range(L1 // 512):
                        nc.tensor.matmul(
                            g[:, it * 512 : (it + 1) * 512],
                            lhsT,
                            tT[:, 2 * b * L1 + it * 512 : 2 * b * L1 + (it + 1) * 512],
                            start=True,
                            stop=True,
                        )
                    nc.scalar.activation(
                        g[:],
                        g[:],
                        AF.Exp,
                        bias=bias_all[:, b * A + jt : b * A + jt + 1],
                        scale=2.0 / SIGMA,
                        accum_out=acc_sb[:, b * A + jt : b * A + jt + 1],
                    )
            nc.sync.dma_start(out=acc_d[:], in_=acc_sb[:])

    _elide_redundant_matmul_waits(nc)
    return nc


def _elide_redundant_matmul_waits(nc):
    """Drop semaphore waits on Matmult instrs that are transitively implied by
    their other waits (Tile emits per-proc-minimal, not transitively-minimal,
    waits; the PE Matmult queue struct only fits one sync wait command).

    Soundness: a wait (S, v) is removed only if chaining (a) same-engine
    in-order start/completion and (b) the completion vector clocks of the
    producers of the REMAINING waits already guarantees S >= v.
    """

    def merge(dst, src):
        for k, v in src.items():
            if dst.get(k, 0) < v:
                dst[k] = v

    all_insts = []
    for bb in nc.bb_map.values():
        all_insts.extend(bb.bb.instructions)
    if True:
        insts = all_insts
        n = len(insts)
        # cumulative updater ticks per semaphore
        sem_updaters = {}  # sem -> list of (cum_value, idx)
        sem_cum = {}
        idx_updates = [[] for _ in range(n)]  # idx -> [(sem, cum_after)]
        for idx, inst in enumerate(insts):
            si = inst.sync_info
            if not si:
                continue
            for u in si.on_update:
                s = u.ant_name
                v = getattr(u, "update_value", None) or 1
                c = sem_cum.get(s, 0) + v
                sem_cum[s] = c
                sem_updaters.setdefault(s, []).append((c, idx))
                idx_updates[idx].append((s, c))

        def producer_of(s, v):
            for c, uidx in sem_updaters.get(s, ()):
                if c >= v:
                    return uidx
            return None

        start_clock = [dict() for _ in range(n)]
        comp_clock = [dict() for _ in range(n)]
        for _ in range(3):
            prev_start = {}
            prev_comp = {}
            for idx, inst in enumerate(insts):
                e = str(inst.engine)
                sc = dict(prev_start.get(e, {}))
                si = inst.sync_info
                if si:
                    for w in si.on_wait:
                        s, v = w.ant_name, w.wait_value
                        if sc.get(s, 0) < v:
                            sc[s] = v
                        p = producer_of(s, v)
                        if p is not None:
                            merge(sc, comp_clock[p])
                cc = dict(sc)
                merge(cc, prev_comp.get(e, {}))
                for s, c in idx_updates[idx]:
                    if cc.get(s, 0) < c:
                        cc[s] = c
                start_clock[idx] = sc
                comp_clock[idx] = cc
                prev_start[e] = sc
                prev_comp[e] = cc

        # elide waits implied by remaining waits + engine order
        prev_start = {}
        for idx, inst in enumerate(insts):
            e = str(inst.engine)
            si = inst.sync_info
            if si and len(si.on_wait) > 1:
                waits = list(si.on_wait)
                kept = list(waits)
                for w in waits:
                    if len(kept) <= 1:
                        break
                    others = [x for x in kept if x is not w]
                    implied = dict(prev_start.get(e, {}))
                    for o in others:
                        if implied.get(o.ant_name, 0) < o.wait_value:
                            implied[o.ant_name] = o.wait_value
                        p = producer_of(o.ant_name, o.wait_value)
                        if p is not None:
                            merge(implied, comp_clock[p])
                    if implied.get(w.ant_name, 0) >= w.wait_value:
                        kept = others
                if len(kept) < len(waits):
                    si.on_wait = kept
                    inst.sync_info = si
            sc = dict(prev_start.get(e, {}))
            if si:
                for w in si.on_wait:
                    if sc.get(w.ant_name, 0) < w.wait_value:
                        sc[w.ant_name] = w.wait_value
                    p = producer_of(w.ant_name, w.wait_value)
                    if p is not None:
                        merge(sc, comp_clock[p])
            prev_start[e] = sc


def _prep(t1, t2, mask1):
    """Build taug (N,4,2,L1) and bias (N,P,A) on host."""
    n1 = np.einsum("nik,nik->ni", t1, t1)  # (N, L1)
    n2 = np.einsum("njk,njk->nj", t2, t2)  # (N, L2)
    with np.errstate(divide="ignore"):
        w1 = -0.5 * n1 + (SIGMA / 2.0) * np.log(mask1)  # -inf where mask==0
    taug = np.empty((N, 4, 2, L1), np.float32)
    taug[:, 0:3, 0, :] = t1.transpose(0, 2, 1)
    taug[:, 3, 0, :] = w1
    taug[:, 0:3, 1, :] = t2.transpose(0, 2, 1)
    taug[:, 3, 1, :] = 1.0
    bias = (-n2 / SIGMA).reshape(N, A, P).transpose(0, 2, 1)  # (N, P, A), j=jt*128+p
    return taug, np.ascontiguousarray(bias, np.float32)


def _make_in_maps(t1, t2, mask1, mask2):
    t1 = np.asarray(t1, dtype=np.float32)
    t2 = np.asarray(t2, dtype=np.float32)
    mask1 = np.asarray(mask1, dtype=np.float32)
    taug, bias = _prep(t1, t2, mask1)
    return [
        {
            "taug": taug[c * NB : (c + 1) * NB],
            "bias": np.ascontiguousarray(
                bias[c * NB : (c + 1) * NB].transpose(1, 0, 2).reshape(P, NB * A)
            ),
        }
        for c in range(NCORES)
    ]


def kernel(t1, t2, mask1, mask2):
    if "nc" not in _CACHE:
        _CACHE["nc"] = _build_program()
    nc = _CACHE["nc"]

    in_maps = _make_in_maps(t1, t2, mask1, mask2)
    res = run_bass_kernel_spmd(nc, in_maps, list(range(NCORES)))

    # per core: acc[p, b*A+jt], j = jt*128+p
    acc = np.stack([r["acc"].reshape(P, NB, A) for r in res.results])  # (C,P,NB,A)
    acc_full = acc.transpose(0, 2, 3, 1).reshape(N, L2).astype(np.float64)

    d = RADIUS + SIGMA * np.log(acc_full + EPSILON)
    d = np.maximum(d, 0.0)
    m2 = np.asarray(mask2).astype(np.float64)
    loss = (d * m2).sum(axis=-1) / m2.sum(axis=-1)
    return loss.astype(np.float32)



# revision 7
# speedup vs baseline: 1.0039x; 1.0039x over previous
"""IntersectionLoss Trainium2 kernel.

Math: loss_n = maskedmean_j relu(R + S*log(sum_i exp(-|t2_nj - t1_ni|^2/S) * m1_i + eps))
Key identity: |t2_j - t1_i|^2 = n2_j + n1_i - 2*t2_j.t1_i, so the inner sum is a
K=4 matmul G = t2aug^T.T @ t1aug^T with augmented rows
  t1aug = [x, y, z, -n1_i/2 + (S/2)ln m1_i],  t2aug = [x, y, z, 1]
(all bf16; n1/n2 are computed from the bf16-rounded coords so the pairwise
distance between rounded points is exact up to the w1-row rounding).

The 8.4M-per-core pairwise exp+reduce is split across two engines:
 - ACT planes: scalar.activation(Exp, scale=2/S, bias=-n2_j/S, accum_out)
   directly on the PSUM plane (1 elem/cycle/lane @1.2GHz).
 - DVE planes: Schraudolph exp-as-bit-trick. tensor_scalar computes
   int16(round(G*SCH_A + schb_j)) whose bits are the bf16 representation of
   exp(x) (z = x*log2e; bits = (z+127-c)*128, c chosen for zero mean error).
   A second tensor_scalar over the bitcast-bf16 tile runs in the DVE 4x
   perf mode and carries the L1-reduction on accum_out.
Each (128 j) x (2048 i) plane is one PE bf16 stream (4 matmuls, one PSUM
bank each); flash-style, the pairwise matrix only ever exists in PSUM.

Sharding: data-parallel over N=16 across 8 cores (2 batches per core).
Final log/relu/masked-mean over the (N,2048) accumulator runs on host.
"""

import sys

sys.path.insert(0, "/opt/trn_rl_repo")

import numpy as np
import ml_dtypes

import concourse.bass as bass
import concourse.tile as tile
from concourse import mybir
from concourse.bass_utils import run_bass_kernel_spmd

RADIUS = 1.0
SIGMA = 2.5
EPSILON = 1e-12

N, L1, L2 = 16, 2048, 2048
NCORES = 8
NB = N // NCORES  # batches per core
P = 128
A = L2 // P  # 16 j-tiles per batch
NPLANES = NB * A  # 32 planes per core
N_DVE = 13  # planes handled by the DVE Schraudolph path (of NPLANES)

F32 = mybir.dt.float32
BF16 = mybir.dt.bfloat16
I16 = mybir.dt.int16
AF = mybir.ActivationFunctionType
OP = mybir.AluOpType

LOG2E = 1.4426950408889634
SCH_C = 0.0575327458840  # zero-mean Schraudolph shift
SCH_A = (2.0 / SIGMA) * LOG2E * 128.0  # scalar1 of DVE pass 1
SCH_B0 = (127.0 - SCH_C) * 128.0  # j-independent part of scalar2

_CACHE = {}


def _dve_planes():
    """Evenly interleaved set of plane indices handled by the DVE path."""
    s = set()
    for idx in range(NPLANES):
        if (idx + 1) * N_DVE // NPLANES > idx * N_DVE // NPLANES:
            s.add(idx)
    return s


def _acc_cols():
    """acc_sb column of each plane: ACT planes pack into the low columns and
    DVE planes into the high ones, so the final DRAM store can be two DMAs
    that each wait on a single producer engine."""
    dve_set = _dve_planes()
    n_act = NPLANES - len(dve_set)
    col_of, a, v = [0] * NPLANES, 0, n_act
    for idx in range(NPLANES):
        if idx in dve_set:
            col_of[idx] = v
            v += 1
        else:
            col_of[idx] = a
            a += 1
    return dve_set, col_of, n_act


def _build_program():
    nc = bass.Bass()
    # taug[b, k, s, i]: s=0 -> t1aug row k, s=1 -> t2aug row k (i in 0..2047)
    taug_d = nc.declare_dram_parameter("taug", (NB, 4, 2, L1), BF16, isOutput=False)
    biasA_d = nc.declare_dram_parameter("biasA", (P, NPLANES), F32, isOutput=False)
    biasV_d = nc.declare_dram_parameter("biasV", (P, NPLANES), F32, isOutput=False)
    acc_d = nc.declare_dram_parameter("acc", (P, NPLANES), F32, isOutput=True)

    dve_set, col_of, n_act = _acc_cols()

    with tile.TileContext(nc) as tc:
        with (
            tc.tile_pool(name="consts", bufs=1) as consts,
            tc.tile_pool(name="sb", bufs=2) as sb,
            tc.tile_pool(name="ei", bufs=2) as eip,
            tc.tile_pool(name="ps", bufs=2, space="PSUM") as ps,
        ):
            # one DMA for all matmul operands: tT[k, (b s i)]
            tT = consts.tile([4, NB * 2 * L1], BF16)
            nc.sync.dma_start(
                out=tT.rearrange("k (b s i) -> k b s i", b=NB, s=2),
                in_=taug_d.rearrange("b k s i -> k b s i"),
            )
            biasA = consts.tile([P, NPLANES], F32)
            nc.sync.dma_start(out=biasA[:], in_=biasA_d[:])
            biasV = consts.tile([P, NPLANES], F32)
            nc.sync.dma_start(out=biasV[:], in_=biasV_d[:])
            # absorb the bias-DMA waits on their consumer engines so the hot
            # instructions carry <=1 wait each
            scrA = consts.tile([P, NPLANES], F32)
            nc.scalar.copy(scrA[:], biasA[:])
            scrV = consts.tile([P, 1], F32)
            nc.vector.tensor_scalar(
                out=scrV[:], in0=biasV[:, 0:1], scalar1=1.0, scalar2=None, op0=OP.mult
            )

            acc_sb = sb.tile([P, NPLANES], F32, tag="acc")
            escr = sb.tile([P, L1], BF16, tag="escr")
            for b in range(NB):
                for jt in range(A):
                    idx = b * A + jt
                    g = ps.tile([P, L1], F32, tag="ps")
                    lhsT = tT[:, (2 * b + 1) * L1 + jt * P : (2 * b + 1) * L1 + (jt + 1) * P]
                    for it in range(L1 // 512):
                        nc.tensor.matmul(
                            g[:, it * 512 : (it + 1) * 512],
                            lhsT,
                            tT[:, 2 * b * L1 + it * 512 : 2 * b * L1 + (it + 1) * 512],
                            start=True,
                            stop=True,
                        )
                    col = slice(col_of[idx], col_of[idx] + 1)
                    if idx in dve_set:
                        ei = eip.tile([P, L1], I16, tag="ei")
                        nc.vector.tensor_scalar(
                            out=ei[:],
                            in0=g[:],
                            scalar1=SCH_A,
                            scalar2=biasV[:, col],
                            op0=OP.mult,
                            op1=OP.add,
                        )
                        nc.vector.tensor_scalar(
                            out=escr[:],
                            in0=ei[:].bitcast(BF16),
                            scalar1=1.0,
                            scalar2=0.0,
                            op0=OP.mult,
                            op1=OP.add,
                            accum_out=acc_sb[:, col],
                        )
                    else:
                        nc.scalar.activation(
                            g[:],
                            g[:],
                            AF.Exp,
                            bias=biasA[:, col],
                            scale=2.0 / SIGMA,
                            accum_out=acc_sb[:, col],
                        )
            nc.sync.dma_start(out=acc_d[:, :n_act], in_=acc_sb[:, :n_act])
            nc.sync.dma_start(out=acc_d[:, n_act:], in_=acc_sb[:, n_act:])

    _elide_redundant_matmul_waits(nc)
    return nc


def _elide_redundant_matmul_waits(nc):
    """Drop semaphore waits on Matmult instrs that are transitively implied by
    their other waits (Tile emits per-proc-minimal, not transitively-minimal,
    waits; the PE Matmult queue struct only fits one sync wait command).

    Soundness: a wait (S, v) is removed only if chaining (a) same-engine
    in-order start/completion and (b) the completion vector clocks of the
    producers of the REMAINING waits already guarantees S >= v.
    """

    def merge(dst, src):
        for k, v in src.items():
            if dst.get(k, 0) < v:
                dst[k] = v

    all_insts = []
    for bb in nc.bb_map.values():
        all_insts.extend(bb.bb.instructions)
    if True:
        insts = all_insts
        n = len(insts)
        # cumulative updater ticks per semaphore
        sem_updaters = {}  # sem -> list of (cum_value, idx)
        sem_cum = {}
        idx_updates = [[] for _ in range(n)]  # idx -> [(sem, cum_after)]
        for idx, inst in enumerate(insts):
            si = inst.sync_info
            if not si:
                continue
            for u in si.on_update:
                s = u.ant_name
                v = getattr(u, "update_value", None) or 1
                c = sem_cum.get(s, 0) + v
                sem_cum[s] = c
                sem_updaters.setdefault(s, []).append((c, idx))
                idx_updates[idx].append((s, c))

        def producer_of(s, v):
            for c, uidx in sem_updaters.get(s, ()):
                if c >= v:
                    return uidx
            return None

        start_clock = [dict() for _ in range(n)]
        comp_clock = [dict() for _ in range(n)]
        for _ in range(3):
            prev_start = {}
            prev_comp = {}
            for idx, inst in enumerate(insts):
                e = str(inst.engine)
                sc = dict(prev_start.get(e, {}))
                si = inst.sync_info
                if si:
                    for w in si.on_wait:
                        s, v = w.ant_name, w.wait_value
                        if sc.get(s, 0) < v:
                            sc[s] = v
                        p = producer_of(s, v)
                        if p is not None:
                            merge(sc, comp_clock[p])
                cc = dict(sc)
                merge(cc, prev_comp.get(e, {}))
                for s, c in idx_updates[idx]:
                    if cc.get(s, 0) < c:
                        cc[s] = c
                start_clock[idx] = sc
                comp_clock[idx] = cc
                prev_start[e] = sc
                prev_comp[e] = cc

        # elide waits implied by remaining waits + engine order
        prev_start = {}
        for idx, inst in enumerate(insts):
            e = str(inst.engine)
            si = inst.sync_info
            if si and len(si.on_wait) > 1:
                waits = list(si.on_wait)
                kept = list(waits)
                for w in waits:
                    if len(kept) <= 1:
                        break
                    others = [x for x in kept if x is not w]
                    implied = dict(prev_start.get(e, {}))
                    for o in others:
                        if implied.get(o.ant_name, 0) < o.wait_value:
                            implied[o.ant_name] = o.wait_value
                        p = producer_of(o.ant_name, o.wait_value)
                        if p is not None:
                            merge(implied, comp_clock[p])
                    if implied.get(w.ant_name, 0) >= w.wait_value:
                        kept = others
                if len(kept) < len(waits):
                    si.on_wait = kept
                    inst.sync_info = si
            sc = dict(prev_start.get(e, {}))
            if si:
                for w in si.on_wait:
                    if sc.get(w.ant_name, 0) < w.wait_value:
                        sc[w.ant_name] = w.wait_value
                    p = producer_of(w.ant_name, w.wait_value)
                    if p is not None:
                        merge(sc, comp_clock[p])
            prev_start[e] = sc
    return nc


def _prep(t1, t2, mask1):
    """Build taug (N,4,2,L1) bf16 and the two bias arrays (N,P,A) f32.

    Coordinates are rounded to bf16 FIRST and n1/n2 computed from the rounded
    values, so the device-side |t2-t1|^2 reconstruction is consistent.
    """
    t1b = t1.astype(ml_dtypes.bfloat16)
    t2b = t2.astype(ml_dtypes.bfloat16)
    t1r = t1b.astype(np.float32)
    t2r = t2b.astype(np.float32)
    n1 = np.einsum("nik,nik->ni", t1r, t1r)  # (N, L1)
    n2 = np.einsum("njk,njk->nj", t2r, t2r)  # (N, L2)
    with np.errstate(divide="ignore"):
        w1 = -0.5 * n1 + (SIGMA / 2.0) * np.log(mask1)
    w1 = np.maximum(w1, -60.0)  # keep the Schraudolph int16 positive
    taug = np.empty((N, 4, 2, L1), ml_dtypes.bfloat16)
    taug[:, 0:3, 0, :] = t1b.transpose(0, 2, 1)
    taug[:, 3, 0, :] = w1.astype(ml_dtypes.bfloat16)
    taug[:, 0:3, 1, :] = t2b.transpose(0, 2, 1)
    taug[:, 3, 1, :] = 1.0
    # j = jt*128 + p  ->  bias[n, p, jt]
    biasA = (-n2 / SIGMA).reshape(N, A, P).transpose(0, 2, 1)
    biasV = (SCH_B0 - n2 * (128.0 * LOG2E / SIGMA)).reshape(N, A, P).transpose(0, 2, 1)
    return (
        taug,
        np.ascontiguousarray(biasA, np.float32),
        np.ascontiguousarray(biasV, np.float32),
    )


def _make_in_maps(t1, t2, mask1, mask2):
    t1 = np.asarray(t1, dtype=np.float32)
    t2 = np.asarray(t2, dtype=np.float32)
    mask1 = np.asarray(mask1, dtype=np.float32)
    taug, biasA, biasV = _prep(t1, t2, mask1)
    maps = []
    for c in range(NCORES):
        sl = slice(c * NB, (c + 1) * NB)
        maps.append(
            {
                "taug": taug[sl],
                "biasA": np.ascontiguousarray(
                    biasA[sl].transpose(1, 0, 2).reshape(P, NPLANES)
                ),
                "biasV": np.ascontiguousarray(
                    biasV[sl].transpose(1, 0, 2).reshape(P, NPLANES)
                ),
            }
        )
    return maps


def kernel(t1, t2, mask1, mask2):
    if "nc" not in _CACHE:
        _CACHE["nc"] = _build_program()
    nc = _CACHE["nc"]

    in_maps = _make_in_maps(t1, t2, mask1, mask2)
    res = run_bass_kernel_spmd(nc, in_maps, list(range(NCORES)))

    # per core: acc[p, col_of[b*A+jt]], j = jt*128+p
    _, col_of, _ = _acc_cols()
    inv = np.argsort(np.asarray(col_of))  # device col -> plane idx? no: plane->col, invert below
    acc = np.stack([r["acc"] for r in res.results])  # (C, P, NPLANES) device cols
    acc_planes = acc[:, :, np.asarray(col_of)]  # (C, P, plane idx = b*A+jt)
    acc_planes = acc_planes.reshape(NCORES, P, NB, A)
    acc_full = acc_planes.transpose(0, 2, 3, 1).reshape(N, L2).astype(np.float64)

    d = RADIUS + SIGMA * np.log(acc_full + EPSILON)
    d = np.maximum(d, 0.0)
    m2 = np.asarray(mask2).astype(np.float64)
    loss = (d * m2).sum(axis=-1) / m2.sum(axis=-1)
    return loss.astype(np.float32)


# revision 10
# speedup vs baseline: 1.0238x; 1.0199x over previous
"""IntersectionLoss Trainium2 kernel.

Math: loss_n = maskedmean_j relu(R + S*log(sum_i exp(-|t2_nj - t1_ni|^2/S) * m1_i + eps))
Key identity: |t2_j - t1_i|^2 = n2_j + n1_i - 2*t2_j.t1_i, so the inner sum is a
K=4 matmul G = t2aug^T.T @ t1aug^T with augmented rows
  t1aug = [x, y, z, -n1_i/2 + (S/2)ln m1_i],  t2aug = [x, y, z, 1]
(all bf16; n1/n2 are computed from the bf16-rounded coords so the pairwise
distance between rounded points is exact up to the w1-row rounding).

The 8.4M-per-core pairwise exp+reduce is split across two engines:
 - ACT planes: scalar.activation(Exp, scale=2/S, bias=-n2_j/S, accum_out)
   directly on the PSUM plane (1 elem/cycle/lane @1.2GHz).
 - DVE planes: Schraudolph exp-as-bit-trick. tensor_scalar computes
   int16(round(G*SCH_A + schb_j)) whose bits are the bf16 representation of
   exp(x) (z = x*log2e; bits = (z+127-c)*128, c chosen for zero mean error).
   A second tensor_scalar over the bitcast-bf16 tile runs in the DVE 4x
   perf mode and carries the L1-reduction on accum_out.
Each (128 j) x (2048 i) plane is one PE bf16 stream (4 matmuls, one PSUM
bank each); flash-style, the pairwise matrix only ever exists in PSUM.

Sharding: data-parallel over N=16 across 8 cores (2 batches per core).
Final log/relu/masked-mean over the (N,2048) accumulator runs on host.
"""

import sys

sys.path.insert(0, "/opt/trn_rl_repo")

import numpy as np
import ml_dtypes

import concourse.bass as bass
import concourse.tile as tile
from concourse import mybir
from concourse.bass_utils import run_bass_kernel_spmd

RADIUS = 1.0
SIGMA = 2.5
EPSILON = 1e-12

N, L1, L2 = 16, 2048, 2048
NCORES = 8
NB = N // NCORES  # batches per core
P = 128
A = L2 // P  # 16 j-tiles per batch
NPLANES = NB * A  # 32 planes per core
N_DVE = 13  # planes handled by the DVE Schraudolph path (of NPLANES)

F32 = mybir.dt.float32
BF16 = mybir.dt.bfloat16
FP8 = mybir.dt.float8e4
I16 = mybir.dt.int16
AF = mybir.ActivationFunctionType
OP = mybir.AluOpType
DR = mybir.MatmulPerfMode.DoubleRow

LOG2E = 1.4426950408889634
SCH_C = 0.0575327458840  # zero-mean Schraudolph shift
SCH_A = (2.0 / SIGMA) * LOG2E * 128.0  # scalar1 of DVE pass 1
SCH_B0 = (127.0 - SCH_C) * 128.0  # j-independent part of scalar2

_CACHE = {}


def _dve_planes():
    """Evenly interleaved set of plane indices handled by the DVE path."""
    s = set()
    for idx in range(NPLANES):
        if (idx + 1) * N_DVE // NPLANES > idx * N_DVE // NPLANES:
            s.add(idx)
    return s


def _acc_cols():
    """acc_sb column of each plane: ACT planes pack into the low columns and
    DVE planes into the high ones, so the final DRAM store can be two DMAs
    that each wait on a single producer engine."""
    dve_set = _dve_planes()
    n_act = NPLANES - len(dve_set)
    col_of, a, v = [0] * NPLANES, 0, n_act
    for idx in range(NPLANES):
        if idx in dve_set:
            col_of[idx] = v
            v += 1
        else:
            col_of[idx] = a
            a += 1
    return dve_set, col_of, n_act


def _build_program():
    nc = bass.Bass()
    # taug[b, k, s, i]: s=0 -> t1aug row k, s=1 -> t2aug row k (i in 0..2047)
    # k rows: [x, y, z, w1a, w1b, 0]; DoubleRow packs them [kp=3, pair=2]
    taug_d = nc.declare_dram_parameter("taug", (NB, 6, 2, L1), FP8, isOutput=False)
    biasA_d = nc.declare_dram_parameter("biasA", (P, NPLANES), F32, isOutput=False)
    biasV_d = nc.declare_dram_parameter("biasV", (P, NPLANES), F32, isOutput=False)
    acc_d = nc.declare_dram_parameter("acc", (P, NPLANES), F32, isOutput=True)

    dve_set, col_of, n_act = _acc_cols()

    with tile.TileContext(nc) as tc:
        with (
            tc.tile_pool(name="consts", bufs=1) as consts,
            tc.tile_pool(name="sb", bufs=2) as sb,
            tc.tile_pool(name="ei", bufs=2) as eip,
            tc.tile_pool(name="yh", bufs=2) as yhp,
            tc.tile_pool(name="ps", bufs=2, space="PSUM") as ps,
        ):
            # one DMA for all matmul operands: tT[kp, pair, (b s i)]
            tT = consts.tile([3, 2, NB * 2 * L1], FP8)
            nc.sync.dma_start(
                out=tT.rearrange("kp pr (b s i) -> kp pr b s i", b=NB, s=2),
                in_=taug_d.rearrange("b (kp pr) s i -> kp pr b s i", kp=3),
            )
            biasA = consts.tile([P, NPLANES], F32)
            nc.sync.dma_start(out=biasA[:], in_=biasA_d[:])
            biasV = consts.tile([P, NPLANES], F32)
            nc.sync.dma_start(out=biasV[:], in_=biasV_d[:])
            # absorb the bias-DMA waits on their consumer engines so the hot
            # instructions carry <=1 wait each
            scrA = consts.tile([P, NPLANES], F32)
            nc.scalar.copy(scrA[:], biasA[:])
            scrV = consts.tile([P, 1], F32)
            nc.vector.tensor_scalar(
                out=scrV[:], in0=biasV[:, 0:1], scalar1=1.0, scalar2=None, op0=OP.mult
            )

            acc_sb = sb.tile([P, NPLANES], F32, tag="acc")
            escr = sb.tile([P, 512], BF16, tag="escr")
            yh2 = sb.tile([P, 512], BF16, tag="yh2")
            for b in range(NB):
                for jt in range(A):
                    idx = b * A + jt
                    g = ps.tile([P, L1], F32, tag="ps")
                    lhsT = tT[:, :, (2 * b + 1) * L1 + jt * P : (2 * b + 1) * L1 + (jt + 1) * P]
                    for it in range(L1 // 512):
                        nc.tensor.matmul(
                            g[:, it * 512 : (it + 1) * 512],
                            lhsT,
                            tT[:, :, 2 * b * L1 + it * 512 : 2 * b * L1 + (it + 1) * 512],
                            start=True,
                            stop=True,
                            perf_mode=DR,
                        )
                    col = slice(col_of[idx], col_of[idx] + 1)
                    if idx in dve_set:
                        ei = eip.tile([P, L1], I16, tag="ei")
                        eb = ei[:].bitcast(BF16)
                        nc.vector.tensor_scalar(
                            out=ei[:],
                            in0=g[:],
                            scalar1=SCH_A,
                            scalar2=biasV[:, col],
                            op0=OP.mult,
                            op1=OP.add,
                        )
                        # reduce: GPS halves 2048->1024, DVE 1024->512, then
                        # a 512-wide accumulate carries the sum to acc_sb
                        yh = yhp.tile([P, 1024], BF16, tag="yh")
                        nc.gpsimd.tensor_tensor(
                            out=yh[:], in0=eb[:, 0:1024], in1=eb[:, 1024:2048], op=OP.add
                        )
                        nc.vector.tensor_tensor(
                            out=yh2[:], in0=yh[:, 0:512], in1=yh[:, 512:1024], op=OP.add
                        )
                        nc.vector.tensor_scalar(
                            out=escr[:],
                            in0=yh2[:],
                            scalar1=1.0,
                            scalar2=0.0,
                            op0=OP.mult,
                            op1=OP.add,
                            accum_out=acc_sb[:, col],
                        )
                    else:
                        nc.scalar.activation(
                            g[:],
                            g[:],
                            AF.Exp,
                            bias=biasA[:, col],
                            scale=2.0 / SIGMA,
                            accum_out=acc_sb[:, col],
                        )
            nc.sync.dma_start(out=acc_d[:, :n_act], in_=acc_sb[:, :n_act])
            nc.sync.dma_start(out=acc_d[:, n_act:], in_=acc_sb[:, n_act:])

    _elide_redundant_matmul_waits(nc)
    return nc


def _elide_redundant_matmul_waits(nc):
    """Drop semaphore waits on Matmult instrs that are transitively implied by
    their other waits (Tile emits per-proc-minimal, not transitively-minimal,
    waits; the PE Matmult queue struct only fits one sync wait command).

    Soundness: a wait (S, v) is removed only if chaining (a) same-engine
    in-order start/completion and (b) the completion vector clocks of the
    producers of the REMAINING waits already guarantees S >= v.
    """

    def merge(dst, src):
        for k, v in src.items():
            if dst.get(k, 0) < v:
                dst[k] = v

    all_insts = []
    for bb in nc.bb_map.values():
        all_insts.extend(bb.bb.instructions)
    if True:
        insts = all_insts
        n = len(insts)
        # cumulative updater ticks per semaphore
        sem_updaters = {}  # sem -> list of (cum_value, idx)
        sem_cum = {}
        idx_updates = [[] for _ in range(n)]  # idx -> [(sem, cum_after)]
        for idx, inst in enumerate(insts):
            si = inst.sync_info
            if not si:
                continue
            for u in si.on_update:
                s = u.ant_name
                v = getattr(u, "update_value", None) or 1
                c = sem_cum.get(s, 0) + v
                sem_cum[s] = c
                sem_updaters.setdefault(s, []).append((c, idx))
                idx_updates[idx].append((s, c))

        def producer_of(s, v):
            for c, uidx in sem_updaters.get(s, ()):
                if c >= v:
                    return uidx
            return None

        start_clock = [dict() for _ in range(n)]
        comp_clock = [dict() for _ in range(n)]
        for _ in range(3):
            prev_start = {}
            prev_comp = {}
            for idx, inst in enumerate(insts):
                e = str(inst.engine)
                sc = dict(prev_start.get(e, {}))
                si = inst.sync_info
                if si:
                    for w in si.on_wait:
                        s, v = w.ant_name, w.wait_value
                        if sc.get(s, 0) < v:
                            sc[s] = v
                        p = producer_of(s, v)
                        if p is not None:
                            merge(sc, comp_clock[p])
                cc = dict(sc)
                merge(cc, prev_comp.get(e, {}))
                for s, c in idx_updates[idx]:
                    if cc.get(s, 0) < c:
                        cc[s] = c
                start_clock[idx] = sc
                comp_clock[idx] = cc
                prev_start[e] = sc
                prev_comp[e] = cc

        # elide waits implied by remaining waits + engine order
        prev_start = {}
        eng_sem_cum = {}  # engine -> {sem: cumulative updates by this engine}
        for idx, inst in enumerate(insts):
            e = str(inst.engine)
            si = inst.sync_info
            if si and len(si.on_wait) > 1:
                waits = list(si.on_wait)
                kept = list(waits)
                # waits on semaphores produced by an EARLIER same-engine
                # instruction are implied by in-order engine execution
                own = eng_sem_cum.get(e, {})
                kept2 = [w for w in kept if own.get(w.ant_name, 0) < w.wait_value]
                if kept2:
                    kept = kept2
                for w in list(kept):
                    if len(kept) <= 1:
                        break
                    others = [x for x in kept if x is not w]
                    implied = dict(prev_start.get(e, {}))
                    for o in others:
                        if implied.get(o.ant_name, 0) < o.wait_value:
                            implied[o.ant_name] = o.wait_value
                        p = producer_of(o.ant_name, o.wait_value)
                        if p is not None:
                            merge(implied, comp_clock[p])
                    if implied.get(w.ant_name, 0) >= w.wait_value:
                        kept = others
                if len(kept) < len(waits):
                    si.on_wait = kept
                    inst.sync_info = si
            sc = dict(prev_start.get(e, {}))
            if si:
                for w in si.on_wait:
                    if sc.get(w.ant_name, 0) < w.wait_value:
                        sc[w.ant_name] = w.wait_value
                    p = producer_of(w.ant_name, w.wait_value)
                    if p is not None:
                        merge(sc, comp_clock[p])
            prev_start[e] = sc
            ec = eng_sem_cum.setdefault(e, {})
            for s, c in idx_updates[idx]:
                if ec.get(s, 0) < c:
                    ec[s] = c
    return nc


def _prep(t1, t2, mask1):
    """Build taug (N,6,2,L1) fp8 and the two bias arrays (N,P,A) f32.

    Coordinates are rounded to fp8 FIRST and n1/n2 computed from the rounded
    values, so the device-side |t2-t1|^2 reconstruction is consistent. The
    -n1/2 row is split into two fp8 rows (value + residual) to keep its
    quantization error second-order.
    """
    f8 = ml_dtypes.float8_e4m3
    t1b = t1.astype(f8)
    t2b = t2.astype(f8)
    t1r = t1b.astype(np.float32)
    t2r = t2b.astype(np.float32)
    n1 = np.einsum("nik,nik->ni", t1r, t1r)  # (N, L1)
    n2 = np.einsum("njk,njk->nj", t2r, t2r)  # (N, L2)
    with np.errstate(divide="ignore"):
        w1 = -0.5 * n1 + (SIGMA / 2.0) * np.log(mask1)
    w1 = np.maximum(w1, -60.0)  # keep the Schraudolph int16 positive
    w1a = w1.astype(f8)
    w1b = (w1 - w1a.astype(np.float32)).astype(f8)
    taug = np.zeros((N, 6, 2, L1), f8)
    taug[:, 0:3, 0, :] = t1b.transpose(0, 2, 1)
    taug[:, 3, 0, :] = w1a
    taug[:, 4, 0, :] = w1b
    taug[:, 0:3, 1, :] = t2b.transpose(0, 2, 1)
    taug[:, 3, 1, :] = 1.0
    taug[:, 4, 1, :] = 1.0
    # j = jt*128 + p  ->  bias[n, p, jt]
    biasA = (-n2 / SIGMA).reshape(N, A, P).transpose(0, 2, 1)
    biasV = (SCH_B0 - n2 * (128.0 * LOG2E / SIGMA)).reshape(N, A, P).transpose(0, 2, 1)
    return (
        taug,
        np.ascontiguousarray(biasA, np.float32),
        np.ascontiguousarray(biasV, np.float32),
    )


def _make_in_maps(t1, t2, mask1, mask2):
    t1 = np.asarray(t1, dtype=np.float32)
    t2 = np.asarray(t2, dtype=np.float32)
    mask1 = np.asarray(mask1, dtype=np.float32)
    taug, biasA, biasV = _prep(t1, t2, mask1)
    maps = []
    for c in range(NCORES):
        sl = slice(c * NB, (c + 1) * NB)
        maps.append(
            {
                "taug": taug[sl],
                "biasA": np.ascontiguousarray(
                    biasA[sl].transpose(1, 0, 2).reshape(P, NPLANES)
                ),
                "biasV": np.ascontiguousarray(
                    biasV[sl].transpose(1, 0, 2).reshape(P, NPLANES)
                ),
            }
        )
    return maps


def kernel(t1, t2, mask1, mask2):
    if "nc" not in _CACHE:
        _CACHE["nc"] = _build_program()
    nc = _CACHE["nc"]

    in_maps = _make_in_maps(t1, t2, mask1, mask2)
    res = run_bass_kernel_spmd(nc, in_maps, list(range(NCORES)))

    # per core: acc[p, col_of[b*A+jt]], j = jt*128+p
    _, col_of, _ = _acc_cols()
    inv = np.argsort(np.asarray(col_of))  # device col -> plane idx? no: plane->col, invert below
    acc = np.stack([r["acc"] for r in res.results])  # (C, P, NPLANES) device cols
    acc_planes = acc[:, :, np.asarray(col_of)]  # (C, P, plane idx = b*A+jt)
    acc_planes = acc_planes.reshape(NCORES, P, NB, A)
    acc_full = acc_planes.transpose(0, 2, 3, 1).reshape(N, L2).astype(np.float64)

    d = RADIUS + SIGMA * np.log(acc_full + EPSILON)
    d = np.maximum(d, 0.0)
    m2 = np.asarray(mask2).astype(np.float64)
    loss = (d * m2).sum(axis=-1) / m2.sum(axis=-1)
    return loss.astype(np.float32)


# revision 11
# speedup vs baseline: 1.0861x; 1.0608x over previous
"""IntersectionLoss Trainium2 kernel.

Math: loss_n = maskedmean_j relu(R + S*log(sum_i exp(-|t2_nj - t1_ni|^2/S) * m1_i + eps))
Key identity: |t2_j - t1_i|^2 = n2_j + n1_i - 2*t2_j.t1_i, so the inner sum is a
K=4 matmul G = t2aug^T.T @ t1aug^T with augmented rows
  t1aug = [x, y, z, -n1_i/2 + (S/2)ln m1_i],  t2aug = [x, y, z, 1]
(all bf16; n1/n2 are computed from the bf16-rounded coords so the pairwise
distance between rounded points is exact up to the w1-row rounding).

The 8.4M-per-core pairwise exp+reduce is split across two engines:
 - ACT planes: scalar.activation(Exp, scale=2/S, bias=-n2_j/S, accum_out)
   directly on the PSUM plane (1 elem/cycle/lane @1.2GHz).
 - DVE planes: Schraudolph exp-as-bit-trick. tensor_scalar computes
   int16(round(G*SCH_A + schb_j)) whose bits are the bf16 representation of
   exp(x) (z = x*log2e; bits = (z+127-c)*128, c chosen for zero mean error).
   A second tensor_scalar over the bitcast-bf16 tile runs in the DVE 4x
   perf mode and carries the L1-reduction on accum_out.
Each (128 j) x (2048 i) plane is one PE bf16 stream (4 matmuls, one PSUM
bank each); flash-style, the pairwise matrix only ever exists in PSUM.

Sharding: data-parallel over N=16 across 8 cores (2 batches per core).
Final log/relu/masked-mean over the (N,2048) accumulator runs on host.
"""

import sys

sys.path.insert(0, "/opt/trn_rl_repo")

import numpy as np
import ml_dtypes

import concourse.bass as bass
import concourse.tile as tile
from concourse import mybir
from concourse.bass_utils import run_bass_kernel_spmd

RADIUS = 1.0
SIGMA = 2.5
EPSILON = 1e-12

N, L1, L2 = 16, 2048, 2048
NCORES = 8
NB = N // NCORES  # batches per core
P = 128
A = L2 // P  # 16 j-tiles per batch
NPLANES = NB * A  # 32 planes per core
N_DVE = 13  # planes handled by the DVE Schraudolph path (of NPLANES)

F32 = mybir.dt.float32
BF16 = mybir.dt.bfloat16
I16 = mybir.dt.int16
AF = mybir.ActivationFunctionType
OP = mybir.AluOpType

LOG2E = 1.4426950408889634
SCH_C = 0.0575327458840  # zero-mean Schraudolph shift
SCH_A = (2.0 / SIGMA) * LOG2E * 128.0  # scalar1 of DVE pass 1
SCH_B0 = (127.0 - SCH_C) * 128.0  # j-independent part of scalar2

_CACHE = {}


def _dve_planes():
    """Evenly interleaved set of plane indices handled by the DVE path."""
    s = set()
    for idx in range(NPLANES):
        if (idx + 1) * N_DVE // NPLANES > idx * N_DVE // NPLANES:
            s.add(idx)
    return s


def _acc_cols():
    """acc_sb column of each plane: ACT planes pack into the low columns and
    DVE planes into the high ones, so the final DRAM store can be two DMAs
    that each wait on a single producer engine."""
    dve_set = _dve_planes()
    n_act = NPLANES - len(dve_set)
    col_of, a, v = [0] * NPLANES, 0, n_act
    for idx in range(NPLANES):
        if idx in dve_set:
            col_of[idx] = v
            v += 1
        else:
            col_of[idx] = a
            a += 1
    return dve_set, col_of, n_act


def _build_program():
    nc = bass.Bass()
    # taug[b, k, s, i]: s=0 -> t1aug row k, s=1 -> t2aug row k (i in 0..2047)
    taug_d = nc.declare_dram_parameter("taug", (NB, 4, 2, L1), BF16, isOutput=False)
    biasA_d = nc.declare_dram_parameter("biasA", (P, NPLANES), F32, isOutput=False)
    biasV_d = nc.declare_dram_parameter("biasV", (P, NPLANES), F32, isOutput=False)
    acc_d = nc.declare_dram_parameter("acc", (P, NPLANES), F32, isOutput=True)

    dve_set, col_of, n_act = _acc_cols()

    with tile.TileContext(nc) as tc:
        with (
            tc.tile_pool(name="consts", bufs=1) as consts,
            tc.tile_pool(name="sb", bufs=2) as sb,
            tc.tile_pool(name="ei", bufs=2) as eip,
            tc.tile_pool(name="yh", bufs=2) as yhp,
            tc.tile_pool(name="ps", bufs=2, space="PSUM") as ps,
        ):
            # matmul operands replicated into the 4 SBUF partition quadrants:
            # row-tile Tq reads its operands from partitions 32q..32q+3
            tT = consts.tile([128, NB * 2 * L1], BF16)
            for q in range(4):
                nc.sync.dma_start(
                    out=tT[32 * q : 32 * q + 4, :].rearrange(
                        "k (b s i) -> k b s i", b=NB, s=2
                    ),
                    in_=taug_d.rearrange("b k s i -> k b s i"),
                )
            biasA = consts.tile([P, NPLANES], F32)
            nc.sync.dma_start(out=biasA[:], in_=biasA_d[:])
            biasV = consts.tile([P, NPLANES], F32)
            nc.sync.dma_start(out=biasV[:], in_=biasV_d[:])
            # absorb the bias-DMA waits on their consumer engines so the hot
            # instructions carry <=1 wait each
            scrA = consts.tile([P, NPLANES], F32)
            nc.scalar.copy(scrA[:], biasA[:])
            scrV = consts.tile([P, 1], F32)
            nc.vector.tensor_scalar(
                out=scrV[:], in0=biasV[:, 0:1], scalar1=1.0, scalar2=None, op0=OP.mult
            )

            acc_sb = sb.tile([P, NPLANES], F32, tag="acc")
            escr = sb.tile([P, 512], BF16, tag="escr")
            yh2 = sb.tile([P, 512], BF16, tag="yh2")
            for b in range(NB):
                for jt in range(A):
                    idx = b * A + jt
                    g = ps.tile([P, L1], F32, tag="ps")
                    lhs_lo = (2 * b + 1) * L1 + jt * P
                    for it in range(L1 // 512):
                        qp = 32 * it
                        nc.tensor.matmul(
                            g[:, it * 512 : (it + 1) * 512],
                            tT[qp : qp + 4, lhs_lo : lhs_lo + P],
                            tT[
                                qp : qp + 4,
                                2 * b * L1 + it * 512 : 2 * b * L1 + (it + 1) * 512,
                            ],
                            start=True,
                            stop=True,
                            tile_position=(qp, 0),
                        )
                    col = slice(col_of[idx], col_of[idx] + 1)
                    if idx in dve_set:
                        ei = eip.tile([P, L1], I16, tag="ei")
                        eb = ei[:].bitcast(BF16)
                        nc.vector.tensor_scalar(
                            out=ei[:],
                            in0=g[:],
                            scalar1=SCH_A,
                            scalar2=biasV[:, col],
                            op0=OP.mult,
                            op1=OP.add,
                        )
                        # reduce: GPS halves 2048->1024, DVE 1024->512, then
                        # a 512-wide accumulate carries the sum to acc_sb
                        yh = yhp.tile([P, 1024], BF16, tag="yh")
                        nc.gpsimd.tensor_tensor(
                            out=yh[:], in0=eb[:, 0:1024], in1=eb[:, 1024:2048], op=OP.add
                        )
                        nc.vector.tensor_tensor(
                            out=yh2[:], in0=yh[:, 0:512], in1=yh[:, 512:1024], op=OP.add
                        )
                        nc.vector.tensor_scalar(
                            out=escr[:],
                            in0=yh2[:],
                            scalar1=1.0,
                            scalar2=0.0,
                            op0=OP.mult,
                            op1=OP.add,
                            accum_out=acc_sb[:, col],
                        )
                    else:
                        nc.scalar.activation(
                            g[:],
                            g[:],
                            AF.Exp,
                            bias=biasA[:, col],
                            scale=2.0 / SIGMA,
                            accum_out=acc_sb[:, col],
                        )
            nc.sync.dma_start(out=acc_d[:, :n_act], in_=acc_sb[:, :n_act])
            nc.sync.dma_start(out=acc_d[:, n_act:], in_=acc_sb[:, n_act:])

    _elide_redundant_matmul_waits(nc)
    return nc


def _elide_redundant_matmul_waits(nc):
    """Drop semaphore waits on Matmult instrs that are transitively implied by
    their other waits (Tile emits per-proc-minimal, not transitively-minimal,
    waits; the PE Matmult queue struct only fits one sync wait command).

    Soundness: a wait (S, v) is removed only if chaining (a) same-engine
    in-order start/completion and (b) the completion vector clocks of the
    producers of the REMAINING waits already guarantees S >= v.
    """

    def merge(dst, src):
        for k, v in src.items():
            if dst.get(k, 0) < v:
                dst[k] = v

    all_insts = []
    for bb in nc.bb_map.values():
        all_insts.extend(bb.bb.instructions)
    if True:
        insts = all_insts
        n = len(insts)
        # cumulative updater ticks per semaphore
        sem_updaters = {}  # sem -> list of (cum_value, idx)
        sem_cum = {}
        idx_updates = [[] for _ in range(n)]  # idx -> [(sem, cum_after)]
        for idx, inst in enumerate(insts):
            si = inst.sync_info
            if not si:
                continue
            for u in si.on_update:
                s = u.ant_name
                v = getattr(u, "update_value", None) or 1
                c = sem_cum.get(s, 0) + v
                sem_cum[s] = c
                sem_updaters.setdefault(s, []).append((c, idx))
                idx_updates[idx].append((s, c))

        def producer_of(s, v):
            for c, uidx in sem_updaters.get(s, ()):
                if c >= v:
                    return uidx
            return None

        start_clock = [dict() for _ in range(n)]
        comp_clock = [dict() for _ in range(n)]
        for _ in range(3):
            prev_start = {}
            prev_comp = {}
            for idx, inst in enumerate(insts):
                e = str(inst.engine)
                sc = dict(prev_start.get(e, {}))
                si = inst.sync_info
                if si:
                    for w in si.on_wait:
                        s, v = w.ant_name, w.wait_value
                        if sc.get(s, 0) < v:
                            sc[s] = v
                        p = producer_of(s, v)
                        if p is not None:
                            merge(sc, comp_clock[p])
                cc = dict(sc)
                merge(cc, prev_comp.get(e, {}))
                for s, c in idx_updates[idx]:
                    if cc.get(s, 0) < c:
                        cc[s] = c
                start_clock[idx] = sc
                comp_clock[idx] = cc
                prev_start[e] = sc
                prev_comp[e] = cc

        # elide waits implied by remaining waits + engine order
        prev_start = {}
        eng_sem_cum = {}  # engine -> {sem: cumulative updates by this engine}
        for idx, inst in enumerate(insts):
            e = str(inst.engine)
            si = inst.sync_info
            if si and len(si.on_wait) > 1:
                waits = list(si.on_wait)
                kept = list(waits)
                # waits on semaphores produced by an EARLIER same-engine
                # instruction are implied by in-order engine execution
                own = eng_sem_cum.get(e, {})
                kept2 = [w for w in kept if own.get(w.ant_name, 0) < w.wait_value]
                if kept2:
                    kept = kept2
                for w in list(kept):
                    if len(kept) <= 1:
                        break
                    others = [x for x in kept if x is not w]
                    implied = dict(prev_start.get(e, {}))
                    for o in others:
                        if implied.get(o.ant_name, 0) < o.wait_value:
                            implied[o.ant_name] = o.wait_value
                        p = producer_of(o.ant_name, o.wait_value)
                        if p is not None:
                            merge(implied, comp_clock[p])
                    if implied.get(w.ant_name, 0) >= w.wait_value:
                        kept = others
                if len(kept) < len(waits):
                    si.on_wait = kept
                    inst.sync_info = si
            sc = dict(prev_start.get(e, {}))
            if si:
                for w in si.on_wait:
                    if sc.get(w.ant_name, 0) < w.wait_value:
                        sc[w.ant_name] = w.wait_value
                    p = producer_of(w.ant_name, w.wait_value)
                    if p is not None:
                        merge(sc, comp_clock[p])
            prev_start[e] = sc
            ec = eng_sem_cum.setdefault(e, {})
            for s, c in idx_updates[idx]:
                if ec.get(s, 0) < c:
                    ec[s] = c
    return nc


def _prep(t1, t2, mask1):
    """Build taug (N,4,2,L1) bf16 and the two bias arrays (N,P,A) f32.

    Coordinates are rounded to bf16 FIRST and n1/n2 computed from the rounded
    values, so the device-side |t2-t1|^2 reconstruction is consistent.
    """
    t1b = t1.astype(ml_dtypes.bfloat16)
    t2b = t2.astype(ml_dtypes.bfloat16)
    t1r = t1b.astype(np.float32)
    t2r = t2b.astype(np.float32)
    n1 = np.einsum("nik,nik->ni", t1r, t1r)  # (N, L1)
    n2 = np.einsum("njk,njk->nj", t2r, t2r)  # (N, L2)
    with np.errstate(divide="ignore"):
        w1 = -0.5 * n1 + (SIGMA / 2.0) * np.log(mask1)
    w1 = np.maximum(w1, -60.0)  # keep the Schraudolph int16 positive
    taug = np.empty((N, 4, 2, L1), ml_dtypes.bfloat16)
    taug[:, 0:3, 0, :] = t1b.transpose(0, 2, 1)
    taug[:, 3, 0, :] = w1.astype(ml_dtypes.bfloat16)
    taug[:, 0:3, 1, :] = t2b.transpose(0, 2, 1)
    taug[:, 3, 1, :] = 1.0
    # j = jt*128 + p  ->  bias[n, p, jt]
    biasA = (-n2 / SIGMA).reshape(N, A, P).transpose(0, 2, 1)
    biasV = (SCH_B0 - n2 * (128.0 * LOG2E / SIGMA)).reshape(N, A, P).transpose(0, 2, 1)
    return (
        taug,
        np.ascontiguousarray(biasA, np.float32),
        np.ascontiguousarray(biasV, np.float32),
    )


def _make_in_maps(t1, t2, mask1, mask2):
    t1 = np.asarray(t1, dtype=np.float32)
    t2 = np.asarray(t2, dtype=np.float32)
    mask1 = np.asarray(mask1, dtype=np.float32)
    taug, biasA, biasV = _prep(t1, t2, mask1)
    maps = []
    for c in range(NCORES):
        sl = slice(c * NB, (c + 1) * NB)
        maps.append(
            {
                "taug": taug[sl],
                "biasA": np.ascontiguousarray(
                    biasA[sl].transpose(1, 0, 2).reshape(P, NPLANES)
                ),
                "biasV": np.ascontiguousarray(
                    biasV[sl].transpose(1, 0, 2).reshape(P, NPLANES)
                ),
            }
        )
    return maps


def kernel(t1, t2, mask1, mask2):
    if "nc" not in _CACHE:
        _CACHE["nc"] = _build_program()
    nc = _CACHE["nc"]

    in_maps = _make_in_maps(t1, t2, mask1, mask2)
    res = run_bass_kernel_spmd(nc, in_maps, list(range(NCORES)))

    # per core: acc[p, col_of[b*A+jt]], j = jt*128+p
    _, col_of, _ = _acc_cols()
    inv = np.argsort(np.asarray(col_of))  # device col -> plane idx? no: plane->col, invert below
    acc = np.stack([r["acc"] for r in res.results])  # (C, P, NPLANES) device cols
    acc_planes = acc[:, :, np.asarray(col_of)]  # (C, P, plane idx = b*A+jt)
    acc_planes = acc_planes.reshape(NCORES, P, NB, A)
    acc_full = acc_planes.transpose(0, 2, 3, 1).reshape(N, L2).astype(np.float64)

    d = RADIUS + SIGMA * np.log(acc_full + EPSILON)
    d = np.maximum(d, 0.0)
    m2 = np.asarray(mask2).astype(np.float64)
    loss = (d * m2).sum(axis=-1) / m2.sum(axis=-1)
    return loss.astype(np.float32)


# revision 14
# speedup vs baseline: 1.0864x; 1.0003x over previous
"""IntersectionLoss Trainium2 kernel.

Math: loss_n = maskedmean_j relu(R + S*log(sum_i exp(-|t2_nj - t1_ni|^2/S) * m1_i + eps))
Key identity: |t2_j - t1_i|^2 = n2_j + n1_i - 2*t2_j.t1_i, so the inner sum is a
K=4 matmul G = t2aug^T.T @ t1aug^T with augmented rows
  t1aug = [x, y, z, -n1_i/2 + (S/2)ln m1_i],  t2aug = [x, y, z, 1]
(all bf16; n1/n2 are computed from the bf16-rounded coords so the pairwise
distance between rounded points is exact up to the w1-row rounding).

The 8.4M-per-core pairwise exp+reduce is split across two engines:
 - ACT planes: scalar.activation(Exp, scale=2/S, bias=-n2_j/S, accum_out)
   directly on the PSUM plane (1 elem/cycle/lane @1.2GHz).
 - DVE planes: Schraudolph exp-as-bit-trick. tensor_scalar computes
   int16(round(G*SCH_A + schb_j)) whose bits are the bf16 representation of
   exp(x) (z = x*log2e; bits = (z+127-c)*128, c chosen for zero mean error).
   A second tensor_scalar over the bitcast-bf16 tile runs in the DVE 4x
   perf mode and carries the L1-reduction on accum_out.
Each (128 j) x (2048 i) plane is one PE bf16 stream (4 matmuls, one PSUM
bank each); flash-style, the pairwise matrix only ever exists in PSUM.

Sharding: data-parallel over N=16 across 8 cores (2 batches per core).
Final log/relu/masked-mean over the (N,2048) accumulator runs on host.
"""

import sys

sys.path.insert(0, "/opt/trn_rl_repo")

import numpy as np
import ml_dtypes

import concourse.bass as bass
import concourse.tile as tile
from concourse import mybir
from concourse.bass_utils import run_bass_kernel_spmd

RADIUS = 1.0
SIGMA = 2.5
EPSILON = 1e-12

N, L1, L2 = 16, 2048, 2048
NCORES = 8
NB = N // NCORES  # batches per core
P = 128
A = L2 // P  # 16 j-tiles per batch
NPLANES = NB * A  # 32 planes per core
N_DVE = 13  # planes handled by the DVE Schraudolph path (of NPLANES)

F32 = mybir.dt.float32
BF16 = mybir.dt.bfloat16
I16 = mybir.dt.int16
AF = mybir.ActivationFunctionType
OP = mybir.AluOpType

LOG2E = 1.4426950408889634
SCH_C = 0.0575327458840  # zero-mean Schraudolph shift
SCH_A = (2.0 / SIGMA) * LOG2E * 128.0  # scalar1 of DVE pass 1
SCH_B0 = (127.0 - SCH_C) * 128.0  # j-independent part of scalar2

_CACHE = {}


def _dve_planes():
    """Evenly interleaved set of plane indices handled by the DVE path."""
    s = set()
    for idx in range(NPLANES):
        if (idx + 1) * N_DVE // NPLANES > idx * N_DVE // NPLANES:
            s.add(idx)
    return s


def _acc_cols():
    """acc_sb column of each plane: ACT planes pack into the low columns and
    DVE planes into the high ones, so the final DRAM store can be two DMAs
    that each wait on a single producer engine."""
    dve_set = _dve_planes()
    n_act = NPLANES - len(dve_set)
    col_of, a, v = [0] * NPLANES, 0, n_act
    for idx in range(NPLANES):
        if idx in dve_set:
            col_of[idx] = v
            v += 1
        else:
            col_of[idx] = a
            a += 1
    return dve_set, col_of, n_act


def _build_program():
    nc = bass.Bass()
    # taug[b, k, s, i]: s=0 -> t1aug row k, s=1 -> t2aug row k (i in 0..2047)
    taug_d = nc.declare_dram_parameter("taug", (NB, 4, 2, L1), BF16, isOutput=False)
    biasA_d = nc.declare_dram_parameter("biasA", (P, NPLANES), F32, isOutput=False)
    biasV_d = nc.declare_dram_parameter("biasV", (P, NPLANES), F32, isOutput=False)
    acc_d = nc.declare_dram_parameter("acc", (P, NPLANES), F32, isOutput=True)

    dve_set, col_of, n_act = _acc_cols()

    with tile.TileContext(nc) as tc:
        with (
            tc.tile_pool(name="consts", bufs=1) as consts,
            tc.tile_pool(name="sb", bufs=2) as sb,
            tc.tile_pool(name="ei", bufs=2) as eip,
            tc.tile_pool(name="yh", bufs=2) as yhp,
            tc.tile_pool(name="ps", bufs=2, space="PSUM") as ps,
        ):
            # matmul operands replicated into the 4 SBUF partition quadrants:
            # row-tile Tq reads its operands from partitions 32q..32q+3
            tT = consts.tile([128, NB * 2 * L1], BF16)
            for q in range(4):
                nc.sync.dma_start(
                    out=tT[32 * q : 32 * q + 4, :].rearrange(
                        "k (b s i) -> k b s i", b=NB, s=2
                    ),
                    in_=taug_d.rearrange("b k s i -> k b s i"),
                )
            biasA = consts.tile([P, NPLANES], F32)
            nc.sync.dma_start(out=biasA[:], in_=biasA_d[:])
            biasV = consts.tile([P, NPLANES], F32)
            nc.sync.dma_start(out=biasV[:], in_=biasV_d[:])
            # absorb the bias-DMA waits on their consumer engines so the hot
            # instructions carry <=1 wait each
            scrA = consts.tile([P, NPLANES], F32)
            nc.scalar.copy(scrA[:], biasA[:])
            scrV = consts.tile([P, 1], F32)
            nc.vector.tensor_scalar(
                out=scrV[:], in0=biasV[:, 0:1], scalar1=1.0, scalar2=None, op0=OP.mult
            )

            acc_sb = sb.tile([P, NPLANES], F32, tag="acc")
            escr = sb.tile([P, 512], BF16, tag="escr")
            yh2 = sb.tile([P, 512], BF16, tag="yh2")

            # deferred second half of a DVE plane's reduction; emitted after
            # the NEXT dve plane's pass-1 so the in-order DVE queue never
            # stalls waiting on the GPS halving of the current plane
            pending = []

            def flush_pending():
                yh_p, col_p = pending.pop()
                nc.vector.tensor_tensor(
                    out=yh2[:], in0=yh_p[:, 0:512], in1=yh_p[:, 512:1024], op=OP.add
                )
                nc.vector.tensor_scalar(
                    out=escr[:],
                    in0=yh2[:],
                    scalar1=1.0,
                    scalar2=0.0,
                    op0=OP.mult,
                    op1=OP.add,
                    accum_out=acc_sb[:, col_p],
                )

            for b in range(NB):
                for jt in range(A):
                    idx = b * A + jt
                    g = ps.tile([P, L1], F32, tag="ps")
                    lhs_lo = (2 * b + 1) * L1 + jt * P
                    for it in range(L1 // 512):
                        qp = 32 * it
                        nc.tensor.matmul(
                            g[:, it * 512 : (it + 1) * 512],
                            tT[qp : qp + 4, lhs_lo : lhs_lo + P],
                            tT[
                                qp : qp + 4,
                                2 * b * L1 + it * 512 : 2 * b * L1 + (it + 1) * 512,
                            ],
                            start=True,
                            stop=True,
                            tile_position=(qp, 0),
                        )
                    col = slice(col_of[idx], col_of[idx] + 1)
                    if idx in dve_set:
                        ei = eip.tile([P, L1], I16, tag="ei")
                        eb = ei[:].bitcast(BF16)
                        nc.vector.tensor_scalar(
                            out=ei[:],
                            in0=g[:],
                            scalar1=SCH_A,
                            scalar2=biasV[:, col],
                            op0=OP.mult,
                            op1=OP.add,
                        )
                        # reduce: GPS halves 2048->1024, DVE 1024->512, then
                        # a 512-wide accumulate carries the sum to acc_sb
                        yh = yhp.tile([P, 1024], BF16, tag="yh")
                        nc.gpsimd.tensor_tensor(
                            out=yh[:], in0=eb[:, 0:1024], in1=eb[:, 1024:2048], op=OP.add
                        )
                        if pending:
                            flush_pending()
                        pending.append((yh, col))
                    else:
                        nc.scalar.activation(
                            g[:],
                            g[:],
                            AF.Exp,
                            bias=biasA[:, col],
                            scale=2.0 / SIGMA,
                            accum_out=acc_sb[:, col],
                        )
            if pending:
                flush_pending()
            nc.sync.dma_start(out=acc_d[:, :n_act], in_=acc_sb[:, :n_act])
            nc.sync.dma_start(out=acc_d[:, n_act:], in_=acc_sb[:, n_act:])

    _elide_redundant_matmul_waits(nc)
    return nc


def _elide_redundant_matmul_waits(nc):
    """Drop semaphore waits on Matmult instrs that are transitively implied by
    their other waits (Tile emits per-proc-minimal, not transitively-minimal,
    waits; the PE Matmult queue struct only fits one sync wait command).

    Soundness: a wait (S, v) is removed only if chaining (a) same-engine
    in-order start/completion and (b) the completion vector clocks of the
    producers of the REMAINING waits already guarantees S >= v.
    """

    def merge(dst, src):
        for k, v in src.items():
            if dst.get(k, 0) < v:
                dst[k] = v

    all_insts = []
    for bb in nc.bb_map.values():
        all_insts.extend(bb.bb.instructions)
    if True:
        insts = all_insts
        n = len(insts)
        # cumulative updater ticks per semaphore
        sem_updaters = {}  # sem -> list of (cum_value, idx)
        sem_cum = {}
        idx_updates = [[] for _ in range(n)]  # idx -> [(sem, cum_after)]
        for idx, inst in enumerate(insts):
            si = inst.sync_info
            if not si:
                continue
            for u in si.on_update:
                s = u.ant_name
                v = getattr(u, "update_value", None) or 1
                c = sem_cum.get(s, 0) + v
                sem_cum[s] = c
                sem_updaters.setdefault(s, []).append((c, idx))
                idx_updates[idx].append((s, c))

        def producer_of(s, v):
            for c, uidx in sem_updaters.get(s, ()):
                if c >= v:
                    return uidx
            return None

        start_clock = [dict() for _ in range(n)]
        comp_clock = [dict() for _ in range(n)]
        for _ in range(3):
            prev_start = {}
            prev_comp = {}
            for idx, inst in enumerate(insts):
                e = str(inst.engine)
                sc = dict(prev_start.get(e, {}))
                si = inst.sync_info
                if si:
                    for w in si.on_wait:
                        s, v = w.ant_name, w.wait_value
                        if sc.get(s, 0) < v:
                            sc[s] = v
                        p = producer_of(s, v)
                        if p is not None:
                            merge(sc, comp_clock[p])
                cc = dict(sc)
                merge(cc, prev_comp.get(e, {}))
                for s, c in idx_updates[idx]:
                    if cc.get(s, 0) < c:
                        cc[s] = c
                start_clock[idx] = sc
                comp_clock[idx] = cc
                prev_start[e] = sc
                prev_comp[e] = cc

        # elide waits implied by remaining waits + engine order
        prev_start = {}
        eng_sem_cum = {}  # engine -> {sem: cumulative updates by this engine}
        for idx, inst in enumerate(insts):
            e = str(inst.engine)
            si = inst.sync_info
            if si and len(si.on_wait) > 1:
                waits = list(si.on_wait)
                kept = list(waits)
                # waits on semaphores produced by an EARLIER same-engine
                # instruction are implied by in-order engine execution
                own = eng_sem_cum.get(e, {})
                kept2 = [w for w in kept if own.get(w.ant_name, 0) < w.wait_value]
                if kept2:
                    kept = kept2
                for w in list(kept):
                    if len(kept) <= 1:
                        break
                    others = [x for x in kept if x is not w]
                    implied = dict(prev_start.get(e, {}))
                    for o in others:
                        if implied.get(o.ant_name, 0) < o.wait_value:
                            implied[o.ant_name] = o.wait_value
                        p = producer_of(o.ant_name, o.wait_value)
                        if p is not None:
                            merge(implied, comp_clock[p])
                    if implied.get(w.ant_name, 0) >= w.wait_value:
                        kept = others
                if len(kept) < len(waits):
                    si.on_wait = kept
                    inst.sync_info = si
            sc = dict(prev_start.get(e, {}))
            if si:
                for w in si.on_wait:
                    if sc.get(w.ant_name, 0) < w.wait_value:
                        sc[w.ant_name] = w.wait_value
                    p = producer_of(w.ant_name, w.wait_value)
                    if p is not None:
                        merge(sc, comp_clock[p])
            prev_start[e] = sc
            ec = eng_sem_cum.setdefault(e, {})
            for s, c in idx_updates[idx]:
                if ec.get(s, 0) < c:
                    ec[s] = c
    return nc


def _prep(t1, t2, mask1):
    """Build taug (N,4,2,L1) bf16 and the two bias arrays (N,P,A) f32.

    Coordinates are rounded to bf16 FIRST and n1/n2 computed from the rounded
    values, so the device-side |t2-t1|^2 reconstruction is consistent.
    """
    t1b = t1.astype(ml_dtypes.bfloat16)
    t2b = t2.astype(ml_dtypes.bfloat16)
    t1r = t1b.astype(np.float32)
    t2r = t2b.astype(np.float32)
    n1 = np.einsum("nik,nik->ni", t1r, t1r)  # (N, L1)
    n2 = np.einsum("njk,njk->nj", t2r, t2r)  # (N, L2)
    with np.errstate(divide="ignore"):
        w1 = -0.5 * n1 + (SIGMA / 2.0) * np.log(mask1)
    w1 = np.maximum(w1, -60.0)  # keep the Schraudolph int16 positive
    taug = np.empty((N, 4, 2, L1), ml_dtypes.bfloat16)
    taug[:, 0:3, 0, :] = t1b.transpose(0, 2, 1)
    taug[:, 3, 0, :] = w1.astype(ml_dtypes.bfloat16)
    taug[:, 0:3, 1, :] = t2b.transpose(0, 2, 1)
    taug[:, 3, 1, :] = 1.0
    # j = jt*128 + p  ->  bias[n, p, jt]
    biasA = (-n2 / SIGMA).reshape(N, A, P).transpose(0, 2, 1)
    biasV = (SCH_B0 - n2 * (128.0 * LOG2E / SIGMA)).reshape(N, A, P).transpose(0, 2, 1)
    return (
        taug,
        np.ascontiguousarray(biasA, np.float32),
        np.ascontiguousarray(biasV, np.float32),
    )


def _make_in_maps(t1, t2, mask1, mask2):
    t1 = np.asarray(t1, dtype=np.float32)
    t2 = np.asarray(t2, dtype=np.float32)
    mask1 = np.asarray(mask1, dtype=np.float32)
    taug, biasA, biasV = _prep(t1, t2, mask1)
    maps = []
    for c in range(NCORES):
        sl = slice(c * NB, (c + 1) * NB)
        maps.append(
            {
                "taug": taug[sl],
                "biasA": np.ascontiguousarray(
                    biasA[sl].transpose(1, 0, 2).reshape(P, NPLANES)
                ),
                "biasV": np.ascontiguousarray(
                    biasV[sl].transpose(1, 0, 2).reshape(P, NPLANES)
                ),
            }
        )
    return maps


def kernel(t1, t2, mask1, mask2):
    if "nc" not in _CACHE:
        _CACHE["nc"] = _build_program()
    nc = _CACHE["nc"]

    in_maps = _make_in_maps(t1, t2, mask1, mask2)
    res = run_bass_kernel_spmd(nc, in_maps, list(range(NCORES)))

    # per core: acc[p, col_of[b*A+jt]], j = jt*128+p
    _, col_of, _ = _acc_cols()
    inv = np.argsort(np.asarray(col_of))  # device col -> plane idx? no: plane->col, invert below
    acc = np.stack([r["acc"] for r in res.results])  # (C, P, NPLANES) device cols
    acc_planes = acc[:, :, np.asarray(col_of)]  # (C, P, plane idx = b*A+jt)
    acc_planes = acc_planes.reshape(NCORES, P, NB, A)
    acc_full = acc_planes.transpose(0, 2, 3, 1).reshape(N, L2).astype(np.float64)

    d = RADIUS + SIGMA * np.log(acc_full + EPSILON)
    d = np.maximum(d, 0.0)
    m2 = np.asarray(mask2).astype(np.float64)
    loss = (d * m2).sum(axis=-1) / m2.sum(axis=-1)
    return loss.astype(np.float32)
